# revision 1
# baseline (speedup 1.0000x reference)
"""Causal self-attention (B=4, T=2048, C=2048, H=16, rope) on 8 trn2 cores.

Sharding: core c handles batch b = c//2 and head-group g = c%2 (8 heads).

Flash-style chunk pipeline, q/k/v SBUF-resident in bf16 (no DRAM round
trip).  T is split into 4 chunks of 512 tokens; per chunk:

  B(ch): qkv for the chunk.  q/k via W-stationary matmuls (N=512) with
    rope fused on DVE straight out of PSUM (sign-vector trick), written
    to SBUF bf16.  v via x-stationary matmuls so it lands pre-transposed
    [t, d] with a ones column appended for the softmax denominator.
  C(ch): causal attention of q-chunk ch over k/v chunks 0..ch per head:
    scoresT = k^T q (bf16, N=512), exp on ACT -> bf16, causal mask
    multiply on diagonal tiles only, attn@V with the ones column,
    per-partition reciprocal normalize, PE-transpose y to [d, t].
  D(ch): per half (4 heads) pairwise AllGather of the y chunk; the last
    chunk's second half goes in two quarter-collectives so the final
    exchange is small.
  E(ch): out[t-chunk] = y^T Wp^T in bf16; all 16 head-tiles come back
    from the AllGather output (replica-indexed, so the program is
    core-independent), accumulation ordered so later collectives land
    at the end of each PSUM chain.

Emission interleaves B(ch+1) | C(ch) | E(ch-1) weighted by per-unit
PE-time so the tensor engine always has dense GEMM work while the
ACT-bound exp stream of C drains.  x and all weights are pre-cast to
bf16 on the host; x is loaded once, with the first chunk's loads split
across four engine queues to cut the startup ramp.
"""
import sys

sys.path.insert(0, "/opt/trn_rl_repo")

import numpy as np
import ml_dtypes

import concourse.bass as bass
import concourse.tile as tile
from concourse import bacc, mybir
from concourse import bass_utils

F32 = mybir.dt.float32
BF16 = mybir.dt.bfloat16
AF = mybir.ActivationFunctionType
ALU = mybir.AluOpType
BF16NP = ml_dtypes.bfloat16

B, T, C = 4, 2048, 2048
NH, D = 16, 128
HL = 8              # heads per core
NCT = C // 128      # 16 c-tiles
NCH = 4             # 512-token chunks
SCALE = 1.0 / np.sqrt(D)
RG = [[0, 1], [2, 3], [4, 5], [6, 7]]


def _weave(*streams):
    """streams: lists of (fn, weight). Emit round-robin by weighted progress."""
    streams = [s for s in streams if s]
    totals = [sum(w for _, w in s) or 1.0 for s in streams]
    done = [0.0] * len(streams)
    idx = [0] * len(streams)
    while True:
        best, bestv = -1, None
        for i, s in enumerate(streams):
            if idx[i] < len(s):
                v = done[i] / totals[i]
                if bestv is None or v < bestv:
                    best, bestv = i, v
        if best < 0:
            break
        fn, w = streams[best][idx[best]]
        fn()
        done[best] += w
        idx[best] += 1


def _build():
    nc = bacc.Bacc("TRN2", target_bir_lowering=False, debug=False, num_devices=8)
    xb = nc.dram_tensor("xb", [128, NCH, 4, 4, 512], BF16, kind="ExternalInput").ap()
    Wqk = nc.dram_tensor("Wqk", [16, 128, C], BF16, kind="ExternalInput").ap()
    WvT = nc.dram_tensor("WvT", [128, 2, NCT, 512], BF16, kind="ExternalInput").ap()
    Wp2 = nc.dram_tensor("Wp2", [128, 2, 2, 8, 512], BF16, kind="ExternalInput").ap()
    cos2 = nc.dram_tensor("cos2", [128, T], BF16, kind="ExternalInput").ap()
    sin1 = nc.dram_tensor("sin1", [64, T], BF16, kind="ExternalInput").ap()
    sgn = nc.dram_tensor("sgn", [128, 1], F32, kind="ExternalInput").ap()
    mask4 = nc.dram_tensor("mask4", [128, 4, 512], BF16, kind="ExternalInput").ap()
    ident = nc.dram_tensor("ident", [128, 128], BF16, kind="ExternalInput").ap()
    out = nc.dram_tensor("out", [T, C // 2], F32, kind="ExternalOutput").ap()

    with tile.TileContext(nc) as tc:
        with tc.tile_pool(name="dram", bufs=1, space="DRAM") as dram, \
             tc.tile_pool(name="const", bufs=1) as cpool:
            ygi = [dram.tile([128, 4, 512], BF16, name=f"ygi{i}") for i in range(7)]
            ygo = [dram.tile([2, 128, 4, 512], BF16, name=f"ygo{i}")
                   for i in range(7)]
            # quarter buffers for the last chunk's second half
            ygiq = [dram.tile([128, 2, 512], BF16, name=f"ygiq{i}") for i in range(2)]
            ygoq = [dram.tile([2, 128, 2, 512], BF16, name=f"ygoq{i}")
                    for i in range(2)]

            xpool = tc.alloc_tile_pool(name="xp", bufs=6)
            wqkp = tc.alloc_tile_pool(name="wqkp", bufs=2)
            wvp = tc.alloc_tile_pool(name="wvp", bufs=1)
            wpp = tc.alloc_tile_pool(name="wpp", bufs=2)
            kp = tc.alloc_tile_pool(name="kp", bufs=32)
            qp = tc.alloc_tile_pool(name="qp", bufs=10)
            vap = tc.alloc_tile_pool(name="vap", bufs=8)
            ebp = tc.alloc_tile_pool(name="ebp", bufs=9)
            yp_ = tc.alloc_tile_pool(name="ypl", bufs=2)
            yfp = tc.alloc_tile_pool(name="yfp", bufs=2)
            abp = tc.alloc_tile_pool(name="abp", bufs=2)
            ynp = tc.alloc_tile_pool(name="ynp", bufs=3)
            op_ = tc.alloc_tile_pool(name="op", bufs=2)
            ps1 = tc.alloc_tile_pool(name="ps1", bufs=2, space="PSUM")
            spp = tc.alloc_tile_pool(name="spp", bufs=2, space="PSUM")
            ypp = tc.alloc_tile_pool(name="ypp", bufs=2, space="PSUM")

            kts = {}   # (h, ch) -> k tile [128, 512]
            qts = {}   # (h, ch) -> q tile [128, 512]
            vas = {}   # (ch, hf) -> va tile [128, 4, 4, 129] (tt, h4, d+1)
            xts = {}   # (ch, qq) -> x tile [128, 4, 512]
            ys = {}    # (ch, hf) -> y tile [128, 4, 512]
            yfs = {}   # (ch, hf) -> yf tile [128, 2, 4, 512] (replica, h4, t)
            wvcur = {}
            wpcur = {}
            ebs = {h: [] for h in range(HL)}

            m4_sb = cpool.tile([128, 4, 512], BF16)
            id_sb = cpool.tile([128, 128], BF16)
            sg_sb = cpool.tile([128, 1], F32)
            c2_sb = cpool.tile([128, T], BF16)
            s1_sb = cpool.tile([64, T], BF16)

            def load_consts():
                nc.scalar.dma_start(c2_sb[:], cos2)
                nc.scalar.dma_start(s1_sb[:], sin1)
                nc.gpsimd.dma_start(sg_sb[:], sgn)
                nc.gpsimd.dma_start(m4_sb[:], mask4)
                nc.gpsimd.dma_start(id_sb[:], ident)

            def load_x(ch, qq):
                def go():
                    xt = xpool.tile([128, 4, 512], BF16, name="xt")
                    if ch == 0:
                        eng = (nc.sync, nc.gpsimd, nc.scalar, nc.sync)[qq]
                    else:
                        eng = nc.sync if qq % 2 == 0 else nc.gpsimd
                    eng.dma_start(xt[:], xb[:, ch, qq])
                    xts[(ch, qq)] = xt
                return go

            def qk_unit(ch, h, kq):
                def go():
                    if ch == 0 and h == 0:
                        # startup: split the first weight loads across two
                        # idle queues
                        wta = wqkp.tile([128, 1024], BF16, name="ws", tag="ws")
                        nc.sync.dma_start(wta[:], Wqk[h * 2 + kq][:, 0:1024])
                        wtb = wqkp.tile([128, 1024], BF16, name="ws", tag="ws")
                        nc.scalar.dma_start(wtb[:], Wqk[h * 2 + kq][:, 1024:2048])
                        halves = (wta, wtb)
                    else:
                        wt = wqkp.tile([128, C], BF16, name="wt")
                        nc.sync.dma_start(wt[:], Wqk[h * 2 + kq])
                        halves = (wt[:, 0:1024], wt[:, 1024:2048])
                    ps = ps1.tile([128, 512], F32, name="ps")
                    for ct in range(NCT):
                        wt = halves[ct // 8]
                        nc.tensor.matmul(
                            ps[:], wt[:, (ct % 8) * 128:(ct % 8 + 1) * 128],
                            xts[(ch, ct // 4)][:, ct % 4, :],
                            start=(ct == 0), stop=(ct == NCT - 1))
                    t0 = ch * 512
                    a_t = abp.tile([128, 512], BF16, name="a_t")
                    nc.vector.tensor_mul(a_t[:], ps[:], c2_sb[:, t0:t0 + 512])
                    b_t = abp.tile([128, 512], BF16, name="b_t")
                    nc.vector.tensor_mul(
                        b_t[0:64, :], ps[64:128, :], s1_sb[:, t0:t0 + 512])
                    nc.vector.tensor_mul(
                        b_t[64:128, :], ps[0:64, :], s1_sb[:, t0:t0 + 512])
                    if kq == 0:
                        dst = qp.tile([128, 512], BF16, name="qt")
                        qts[(h, ch)] = dst
                    else:
                        dst = kp.tile([128, 512], BF16, name="kt")
                        kts[(h, ch)] = dst
                    nc.vector.scalar_tensor_tensor(
                        dst[:], b_t[:], sg_sb[:], a_t[:],
                        op0=ALU.mult, op1=ALU.add)
                return go

            def v_unit(ch, hf, tt):
                def go():
                    if tt == 0:
                        wv = wvp.tile([128, NCT, 512], BF16, name="wv")
                        nc.sync.dma_start(wv[:], WvT[:, hf])
                        wvcur[0] = wv
                        va = vap.tile([128, 4, 4, 129], BF16, name="va")
                        nc.vector.memset(va[:, :, :, 128:129], 1.0)
                        vas[(ch, hf)] = va
                    wv = wvcur[0]
                    va = vas[(ch, hf)]
                    vps = ps1.tile([128, 512], F32, name="ps")
                    for ct in range(NCT):
                        nc.tensor.matmul(
                            vps[:],
                            xts[(ch, ct // 4)][:, ct % 4, tt * 128:(tt + 1) * 128],
                            wv[:, ct, :],
                            start=(ct == 0), stop=(ct == NCT - 1))
                    nc.scalar.copy(
                        va[:, tt, :, 0:128],
                        vps.rearrange("p (h d) -> p h d", d=128))
                return go

            def s_unit(ch, h, b2):
                def go():
                    sp = spp.tile([128, 2, 512], F32, name="sp")
                    for jj in range(2):
                        j = 2 * b2 + jj
                        nc.tensor.matmul(
                            sp[:, jj, :],
                            kts[(h, j // 4)][:, (j % 4) * 128:(j % 4 + 1) * 128],
                            qts[(h, ch)][:],
                            start=True, stop=True)
                    eb = ebp.tile([128, 2, 512], BF16, name="eb")
                    nc.scalar.activation(eb[:], sp[:], AF.Exp, scale=float(SCALE))
                    if b2 == 2 * ch:
                        nc.vector.tensor_mul(eb[:], eb[:], m4_sb[:, 0:2, :])
                    elif b2 == 2 * ch + 1:
                        nc.vector.tensor_mul(eb[:], eb[:], m4_sb[:, 2:4, :])
                    ebs[h].append(eb)
                return go

            def a_unit(ch, h, ql):
                def go():
                    yt = ypp.tile([128, 129], F32, name="yp", tag="yp")
                    jmax = 4 * ch + ql
                    for j in range(jmax + 1):
                        nc.tensor.matmul(
                            yt[:],
                            ebs[h][j // 2][:, j % 2, ql * 128:(ql + 1) * 128],
                            vas[(j // 4, h // 4)][:, j % 4, h % 4, :],
                            start=(j == 0), stop=(j == jmax))
                    rc = ynp.tile([128, 1], F32, name="rc")
                    nc.vector.reciprocal(rc[:], yt[:, 128:129])
                    yn = ynp.tile([128, 128], BF16, name="yn")
                    nc.vector.tensor_scalar_mul(yn[:], yt[:, 0:128], rc[:])
                    ytp = ypp.tile([128, 128], BF16, name="ytp", tag="yp")
                    nc.tensor.transpose(ytp[:], yn[:], id_sb[:])
                    nc.vector.tensor_copy(
                        ys[(ch, h // 4)][:, h % 4, ql * 128:(ql + 1) * 128],
                        ytp[:])
                return go

            def y_alloc(ch, hf):
                def go():
                    ys[(ch, hf)] = yp_.tile([128, 4, 512], BF16, name="yc")
                return go

            def d_unit(ch, hf):
                def go():
                    i = ch * 2 + hf
                    nc.gpsimd.dma_start(ygi[i], ys[(ch, hf)][:])
                    nc.gpsimd.collective_compute(
                        "AllGather", ALU.bypass,
                        ins=[ygi[i][:].opt()], outs=[ygo[i][:].opt()],
                        replica_groups=RG)
                return go

            def yf_load(ch, hf):
                def go():
                    yf = yfp.tile([128, 2, 4, 512], BF16, name="yf")
                    nc.sync.dma_start(
                        yf[:], ygo[ch * 2 + hf].rearrange("r p h t -> p r h t"))
                    yfs[(ch, hf)] = yf
                return go

            def d_quarter(qi):
                """Quarter exchange for (ch=3, hf=1): qi=0 -> heads 4,5;
                qi=1 -> heads 6,7."""
                def go():
                    nc.gpsimd.dma_start(
                        ygiq[qi], ys[(3, 1)][:, 2 * qi:2 * qi + 2, :])
                    nc.gpsimd.collective_compute(
                        "AllGather", ALU.bypass,
                        ins=[ygiq[qi][:].opt()], outs=[ygoq[qi][:].opt()],
                        replica_groups=RG)
                    if qi == 0:
                        yf = yfp.tile([128, 2, 4, 512], BF16, name="yf")
                        yfs[(3, 1)] = yf
                    yf = yfs[(3, 1)]
                    nc.sync.dma_start(
                        yf[:, :, 2 * qi:2 * qi + 2, :],
                        ygoq[qi].rearrange("r p h t -> p r h t"))
                return go

            def wp_load(ch, fc):
                """Prefetch proj weights for (ch, fc). For the final
                (ch=3, fc=1) borrow idle x-pool slots so the load never
                waits on fc0's readers."""
                def go():
                    if ch == 3 and fc == 1:
                        for r in range(2):
                            for hh in range(2):
                                wph = xpool.tile([128, 4, 512], BF16, name="xt")
                                (nc.sync if hh == 0 else nc.gpsimd).dma_start(
                                    wph[:], Wp2[:, r, fc, 4 * hh:4 * hh + 4])
                                wpcur[(fc, r, hh)] = wph
                    else:
                        for r in range(2):
                            wp = wpp.tile([128, 8, 512], BF16, name="wp")
                            nc.sync.dma_start(wp[:], Wp2[:, r, fc])
                            wpcur[(fc, r, 0)] = wp[:, 0:4, :]
                            wpcur[(fc, r, 1)] = wp[:, 4:8, :]
                return go

            def e_unit(ch, fc, tt):
                def go():
                    pp = ps1.tile([128, 512], F32, name="ps")
                    if ch == 3:
                        # late-arriving quarters last: hf0 r0/r1, hf1[h01]
                        # r0/r1, hf1[h23] r0/r1
                        srcs = [(0, r, h4) for r in range(2) for h4 in range(4)]
                        srcs += [(1, r, h4) for h4g in range(2) for r in range(2)
                                 for h4 in (2 * h4g, 2 * h4g + 1)]
                    else:
                        srcs = [(hf, r, h4) for hf in range(2) for r in range(2)
                                for h4 in range(4)]
                    for i, (hf, r, h4) in enumerate(srcs):
                        wp = wpcur[(fc, r, hf)]
                        nc.tensor.matmul(
                            pp[:],
                            yfs[(ch, hf)][:, r, h4, tt * 128:(tt + 1) * 128],
                            wp[:, h4, :],
                            start=(i == 0), stop=(i == 15))
                    ob = op_.tile([128, 512], F32, name="ob")
                    nc.vector.tensor_copy(ob[:], pp[:])
                    t0 = ch * 512 + tt * 128
                    nc.sync.dma_start(
                        out[t0:t0 + 128, fc * 512:(fc + 1) * 512], ob[:])
                return go

            def b_stream(ch, with_xl=None):
                u = []
                for h in (0, 1):
                    u.append((qk_unit(ch, h, 0), 3.4))
                    u.append((qk_unit(ch, h, 1), 3.4))
                for tt in range(4):
                    u.append((v_unit(ch, 0, tt), 3.4))
                for h in (2, 3):
                    u.append((qk_unit(ch, h, 0), 3.4))
                    u.append((qk_unit(ch, h, 1), 3.4))
                for tt in range(4):
                    u.append((v_unit(ch, 1, tt), 3.4))
                for h in (4, 5, 6, 7):
                    u.append((qk_unit(ch, h, 0), 3.4))
                    u.append((qk_unit(ch, h, 1), 3.4))
                if with_xl is not None:
                    for qi in range(4):
                        u.append((load_x(with_xl, qi), 0.1))
                return u

            def c_stream(ch):
                u = []
                for h in range(8):
                    if h % 4 == 0:
                        u.append((y_alloc(ch, h // 4), 0.05))

                    def reset(h=h):
                        ebs[h] = []
                    u.append((reset, 0.0))
                    for b2 in range(2 * ch + 2):
                        u.append((s_unit(ch, h, b2), 0.9))
                    for ql in range(4):
                        u.append((a_unit(ch, h, ql), 0.4 + (4 * ch + ql) * 0.066))
                    if h == 3:
                        u.append((d_unit(ch, 0), 0.1))
                    elif h == 7:
                        if ch == 3:
                            u.append((d_quarter(1), 0.1))
                        else:
                            u.append((d_unit(ch, 1), 0.1))
                    elif h == 5 and ch == 3:
                        u.append((d_quarter(0), 0.1))
                return u

            def e_stream(ch):
                u = [(wp_load(ch, 0), 0.1)]
                u.append((yf_load(ch, 0), 0.1))
                if ch < 3:
                    u.append((yf_load(ch, 1), 0.1))
                for tt in range(4):
                    u.append((e_unit(ch, 0, tt), 3.5))
                u.append((wp_load(ch, 1), 0.1))
                for tt in range(4):
                    u.append((e_unit(ch, 1, tt), 3.5))
                return u

            # ---------------- emit ----------------
            for qi in range(4):
                load_x(0, qi)()
            load_consts()
            for fn, _ in b_stream(0, with_xl=1):
                fn()
            _weave(b_stream(1, with_xl=2), c_stream(0))
            _weave(b_stream(2, with_xl=3), c_stream(1), e_stream(0))
            _weave(b_stream(3), c_stream(2), e_stream(1))
            e3 = e_stream(3)
            _weave(c_stream(3) + [e3[0], e3[1], e3[6]], e_stream(2))
            for fn, _ in e3[2:6] + e3[7:]:
                fn()

            for p in [ypp, spp, ps1, op_, ynp, abp, yfp, yp_, ebp, vap, qp,
                      kp, wpp, wvp, wqkp, xpool]:
                p.release()
    nc.compile()
    return nc


_NC = None


def _get_nc():
    global _NC
    if _NC is None:
        _NC = _build()
    return _NC


def _rope_tables():
    inv_freq = (1.0 / (10000.0 ** (np.arange(0, D, 2, dtype=np.float32) / D)))
    t = np.arange(T, dtype=np.float32)
    freqs = np.outer(t, inv_freq).astype(np.float32)      # [T, 64]
    cos = np.cos(freqs).T                                 # [64, T]
    sin = np.sin(freqs).T
    return cos, sin


def _tile_w(Wt):
    """[128 r, 2048 c] weight tile -> [128 c_lo, 2048 (ct r)] layout."""
    return np.ascontiguousarray(
        Wt.T.reshape(NCT, 128, 128).transpose(1, 0, 2).reshape(128, C))


def make_in_maps(x, W_attn, W_proj):
    perm = np.concatenate([np.arange(0, D, 2), np.arange(1, D, 2)])
    cos, sin = _rope_tables()
    cos2 = np.concatenate([cos, cos], 0).astype(BF16NP)
    sin1 = np.ascontiguousarray(sin).astype(BF16NP)
    sgn = np.concatenate([-np.ones((64, 1)), np.ones((64, 1))]).astype(np.float32)
    p_i = np.arange(128)[:, None, None]
    jj_i = np.arange(4)[None, :, None]
    c_i = np.arange(512)[None, None, :]
    mask4 = (c_i >= p_i + 128 * jj_i).astype(BF16NP)

    xbf = x.astype(BF16NP)
    in_maps = []
    for core in range(8):
        b, g = core // 2, core % 2
        # xb [128 p, ch, qq, ct4, 512 t] = x[b, ch*512+t, (qq*4+ct4)*128+p]
        xt = np.ascontiguousarray(xbf[b].T)               # [C, T]
        xb_ = xt.reshape(4, 4, 128, NCH, 512).transpose(2, 3, 0, 1, 4)
        # q/k weight tiles, rope-permuted; order [h0 q, h0 k, h1 q, ...]
        wtiles = []
        for h in range(HL):
            hg = g * HL + h
            wtiles.append(_tile_w(W_attn[hg * D:(hg + 1) * D][perm]))
            wtiles.append(_tile_w(W_attn[C + hg * D:C + (hg + 1) * D][perm]))
        Wqk_ = np.stack(wtiles, 0).astype(BF16NP)
        # WvT [128 p, half, ct, 512 dv] = Wv[g*1024 + hf*512 + dv, ct*128 + p]
        wv = W_attn[2 * C + g * 1024:2 * C + (g + 1) * 1024]   # [1024 dv, C]
        WvT_ = np.ascontiguousarray(
            wv.reshape(2, 512, NCT, 128).transpose(3, 0, 2, 1)).astype(BF16NP)
        # Wp2 [128 p, r, fc, 8, 512 f] = Wp[g*1024 + fc*512 + f, (r*8+i8)*128+p]
        wp = W_proj[g * 1024:(g + 1) * 1024]                   # [1024 f, C]
        wp_t = wp.reshape(2, 512, NCT, 128)                    # [fc, f, ct, p]
        Wp2_ = np.stack([wp_t[:, :, 0:8], wp_t[:, :, 8:16]], 0)  # [r, fc, f, 8, p]
        Wp2_ = np.ascontiguousarray(Wp2_.transpose(4, 0, 1, 3, 2)).astype(BF16NP)
        in_maps.append({
            "xb": np.ascontiguousarray(xb_),
            "Wqk": Wqk_,
            "WvT": WvT_,
            "Wp2": Wp2_,
            "cos2": cos2, "sin1": sin1, "sgn": sgn,
            "mask4": mask4, "ident": np.eye(128, dtype=BF16NP),
        })
    return in_maps


def _assemble(results):
    out = np.empty((B, T, C), dtype=np.float32)
    for core in range(8):
        b, g = core // 2, core % 2
        out[b][:, g * (C // 2):(g + 1) * (C // 2)] = results[core]["out"]
    return out


def run(x, W_attn, W_proj, **spmd_kwargs):
    nc = _get_nc()
    in_maps = make_in_maps(np.asarray(x, dtype=np.float32),
                           np.asarray(W_attn, dtype=np.float32),
                           np.asarray(W_proj, dtype=np.float32))
    res = bass_utils.run_bass_kernel_spmd(
        nc, in_maps, core_ids=list(range(8)), **spmd_kwargs)
    return _assemble(res.results), res


def kernel(x, W_attn, W_proj):
    out, _ = run(x, W_attn, W_proj)
    return out

